# revision 28
# baseline (speedup 1.0000x reference)
"""Multi-head attention (B=2, S=2048, D=1024, H=16) on 8 trn2 NeuronCores.

Sharding: core c handles batch b = c // 4 and head-group g = c % 4
(4 heads = 256 hidden columns per core).  Each core computes its 4 heads'
attention plus the partial out-projection; the host sums the 4 partials
per batch and adds the (linear) bias terms (bo + Wo @ bv) exactly.

v2: all matmuls in bf16 (1 cycle/row on the PE; fp32 accumulation in
PSUM), exp in [128,1024] tiles, reciprocal_approx_fast for softmax
denominators.

Layout per core (DRAM tensors bf16 unless noted):
  xqT, xkT, xvT : [1024, 2048]   x.T (host-transposed activations)
  wqT, wkT, wvT : [1024, 256]    W.T column slice for this head group
  woT           : [256, 1024]    Wo[:, J].T
  bq, bk        : [256]  fp32    bias slices (added via ACT during evac)
  outT (output) : [1024, 2048] fp32   partial (out @ Wo_J.T).T
"""
import os
import sys
import types

sys.path.insert(0, "/opt/trn_rl_repo")

import numpy as np

B = 2
S = 2048
D = 1024
H_PER_CORE = 4      # heads per core
DH = 64             # head dim
JG = 256            # hidden cols per core (4 heads * 64)
ND = D // 128       # 8 contraction d-tiles
NKT = S // 128      # 16 k-position tiles
QC = 512
PC = 1024           # processed q columns per pass (2 chunks of 512)
SCALE = 1.0 / np.sqrt(DH)

_cache = {}


def _install_profshim():
    """Enable NTFF profiling under axon (KERNEL_TRACE=1 only)."""
    if "antenv.axon_hooks" in sys.modules:
        return
    try:
        from trn_agent_boot.trn_boot import _ntff_profile_via_ctypes

        hook = _ntff_profile_via_ctypes("/opt/axon/libaxon_pjrt.so")
        mod = types.ModuleType("antenv.axon_hooks")
        mod.get_axon_ntff_profile_hook = lambda: hook
        mod.set_axon_ntff_profile_hook = lambda h: None
        sys.modules["antenv.axon_hooks"] = mod
        import concourse.bass_utils as _bu

        _bu.upload_artifacts = lambda tmpdir: "local://unavailable"
    except Exception:
        pass


def build_nc():
    import concourse.bacc as bacc
    import concourse.mybir as mybir
    import concourse.tile as tile

    f32 = mybir.dt.float32
    bf16 = mybir.dt.bfloat16
    AF = mybir.ActivationFunctionType

    nc = bacc.Bacc("TRN2", target_bir_lowering=False)

    xqT = nc.dram_tensor("xqT", [D, S], bf16, kind="ExternalInput").ap()
    xkT = nc.dram_tensor("xkT", [D, S], bf16, kind="ExternalInput").ap()
    xvT = nc.dram_tensor("xvT", [D, S], bf16, kind="ExternalInput").ap()
    wqT = nc.dram_tensor("wqT", [D, JG], bf16, kind="ExternalInput").ap()
    wkT = nc.dram_tensor("wkT", [D, JG], bf16, kind="ExternalInput").ap()
    wvT = nc.dram_tensor("wvT", [D, JG], bf16, kind="ExternalInput").ap()
    woT = nc.dram_tensor("woT", [JG, D], bf16, kind="ExternalInput").ap()
    bq = nc.dram_tensor("bq", [JG], f32, kind="ExternalInput").ap()
    bk = nc.dram_tensor("bk", [JG], f32, kind="ExternalInput").ap()
    outT = nc.dram_tensor("outT", [D, S], f32, kind="ExternalOutput").ap()

    with tile.TileContext(nc) as tc:
        with (
            tc.tile_pool(name="xt", bufs=8) as xt_pool,
            tc.tile_pool(name="wts", bufs=1) as w_pool,
            tc.tile_pool(name="qkv", bufs=1) as qkv_pool,
            tc.tile_pool(name="attn", bufs=2) as attn_pool,
            tc.tile_pool(name="small", bufs=1) as small_pool,
            tc.tile_pool(name="nrm", bufs=2) as nrm_pool,
            tc.tile_pool(name="oev", bufs=3) as oev_pool,
        ):
            # ---- weight / bias loads -------------------------------------
            # wq first so the very first projection matmul can start as
            # soon as wq + the first xq tile land; the rest follow behind
            wq_t = w_pool.tile([128, ND, JG], bf16, tag="wq")
            nc.sync.dma_start(wq_t[:], wqT.rearrange("(n p) j -> p n j", p=128))
            wk_t = w_pool.tile([128, ND, JG], bf16, tag="wk")
            wv_t = w_pool.tile([128, ND, JG], bf16, tag="wv")
            wo_t = w_pool.tile([128, 2, ND, 128], bf16, tag="wo")
            bq_t = small_pool.tile([128, 2], f32, tag="bq")
            bk_t = small_pool.tile([128, 2], f32, tag="bk")
            ones1 = small_pool.tile([1, DH], f32, tag="ones1")
            nc.vector.memset(ones1[:], 1.0)

            def load_rest_of_weights():
                nc.sync.dma_start(
                    wk_t[:], wkT.rearrange("(n p) j -> p n j", p=128)
                )
                nc.sync.dma_start(
                    wv_t[:], wvT.rearrange("(n p) j -> p n j", p=128)
                )
                nc.sync.dma_start(
                    wo_t[:],
                    woT.rearrange("(a p) (n m) -> p a n m", p=128, m=128),
                )
                nc.sync.dma_start(bq_t[:], bq.rearrange("(a p) -> p a", p=128))
                nc.sync.dma_start(bk_t[:], bk.rearrange("(a p) -> p a", p=128))

            # ---- persistent activation tensors ---------------------------
            q_t = [qkv_pool.tile([128, S], bf16, tag=f"qt{m}", name=f"qt{m}")
                   for m in range(2)]
            k_t = [qkv_pool.tile([128, S], bf16, tag=f"kt{m}", name=f"kt{m}")
                   for m in range(2)]
            # V (natural layout) + ones column per head: 16 s-tiles
            v_t = [qkv_pool.tile([128, H_PER_CORE, DH + 1], bf16,
                                 tag=f"v{s}", name=f"v{s}")
                   for s in range(NKT)]
            ao_t = [qkv_pool.tile([128, S], bf16, tag=f"ao{m}", name=f"ao{m}")
                    for m in range(2)]

            # ---- phase 1: projections (d-outer, PSUM-resident) -----------
            with tc.tile_pool(name="proj_psum", bufs=1, space="PSUM") as pp:
                for name, w_full, x_dram, dst, bias in (
                    ("q", wq_t, xqT, q_t, bq_t),
                    ("k", wk_t, xkT, k_t, bk_t),
                ):
                    xs = []
                    for d in range(ND):
                        xd = xt_pool.tile([128, S], bf16, tag="xT",
                                          name=f"x{name}{d}")
                        nc.sync.dma_start(
                            xd[:], x_dram[d * 128:(d + 1) * 128, :]
                        )
                        xs.append(xd)
                    if name == "q":
                        load_rest_of_weights()
                    ps = {
                        (m, c): pp.tile([128, QC], f32, tag=f"pp{m}{c}",
                                        name=f"ps{name}{m}{c}")
                        for m in range(2) for c in range(4)
                    }
                    for d in range(ND):
                        for m in range(2):
                            for c in range(4):
                                nc.tensor.matmul(
                                    ps[(m, c)][:],
                                    w_full[:, d, m * 128:(m + 1) * 128],
                                    xs[d][:, c * QC:(c + 1) * QC],
                                    start=(d == 0),
                                    stop=(d == ND - 1),
                                )
                    for m in range(2):
                        for c in range(4):
                            nc.vector.tensor_scalar_add(
                                dst[m][:, c * QC:(c + 1) * QC],
                                ps[(m, c)][:],
                                bias[:, m:m + 1],
                            )

                # xv tiles stream in behind the K loads
                xvs = []
                for d in range(ND):
                    xd = xt_pool.tile([128, S], bf16, tag="xT", name=f"xv{d}")
                    nc.sync.dma_start(xd[:], xvT[d * 128:(d + 1) * 128, :])
                    xvs.append(xd)

            # ---- phase 2+3: attention + V + out-projection ---------------
            # The first head-pair's score/exp blocks are emitted BEFORE the
            # V projection so the ACT exp queue drains while the PE runs V.
            with tc.tile_pool(name="apsum", bufs=1, space="PSUM") as ap_pool:

                def sc_exp_block(p, hp, hh):
                    pc0 = p * PC
                    po = hh * DH
                    ats = []
                    for kt in range(NKT):
                        sc_ps = ap_pool.tile(
                            [128, PC], f32, tag=f"sc{kt % 2}",
                            name=f"sc_{p}{hp}{hh}_{kt}",
                        )
                        for n in range(2):
                            nc.tensor.matmul(
                                sc_ps[:, n * QC:(n + 1) * QC],
                                k_t[hp][po:po + DH,
                                        kt * 128:(kt + 1) * 128],
                                q_t[hp][po:po + DH,
                                        pc0 + n * QC:pc0 + (n + 1) * QC],
                                start=True, stop=True,
                            )
                        at = attn_pool.tile(
                            [128, PC], bf16, tag=f"at{kt}",
                            name=f"at{p}{hp}{hh}_{kt}",
                        )
                        nc.scalar.activation(
                            at[:], sc_ps[:], AF.Exp, scale=float(SCALE)
                        )
                        ats.append(at)
                    return ats

                def av_block(p, hp, hh, ats):
                    h = hp * 2 + hh
                    av = ap_pool.tile(
                        [DH + 1, PC], f32, tag=f"av{hh}",
                        name=f"av{hh}_{hp}_{p}",
                    )
                    for kt in range(NKT):
                        for n in range(2):
                            nc.tensor.matmul(
                                av[:, n * QC:(n + 1) * QC],
                                v_t[kt][:, h, :],
                                ats[kt][:, n * QC:(n + 1) * QC],
                                start=(kt == 0),
                                stop=(kt == NKT - 1),
                            )
                    return av

                def norm_block(p, hp, hh, av):
                    # PE-free normalize: recip on DVE, partition broadcast
                    # on GPSIMD, multiply on DVE
                    psl = slice(p * PC, (p + 1) * PC)
                    po = hh * DH
                    dn = nrm_pool.tile([1, PC], f32, tag="dn",
                                       name=f"dn{p}{hp}{hh}")
                    nc.vector.tensor_copy(dn[:], av[DH:DH + 1, :])
                    rc = nrm_pool.tile([1, PC], f32, tag="rc",
                                       name=f"rc{p}{hp}{hh}")
                    nc.vector.reciprocal_approx_fast(rc[:], dn[:])
                    rb = nrm_pool.tile([DH, PC], f32, tag="rb",
                                       name=f"rb{p}{hp}{hh}")
                    nc.gpsimd.partition_broadcast(rb[:], rc[:])
                    nc.vector.tensor_mul(
                        ao_t[hp][po:po + DH, psl], av[0:DH, :], rb[:]
                    )

                # early scores for (p0, hp0) — fills the ACT pipe
                ats_early = {hh: sc_exp_block(0, 0, hh) for hh in range(2)}

                # V projection (PSUM slots borrowed from the av tags),
                # 8 waves of 2 s-tiles
                ones4 = small_pool.tile([128, H_PER_CORE], f32, tag="ones4")
                nc.vector.memset(ones4[:], 1.0)
                for w in range(8):
                    ps = {
                        s: ap_pool.tile([128, JG], f32, tag=f"av{s % 2}",
                                        name=f"psv{s}")
                        for s in (2 * w, 2 * w + 1)
                    }
                    for d in range(ND):
                        for s in (2 * w, 2 * w + 1):
                            nc.tensor.matmul(
                                ps[s][:],
                                xvs[d][:, s * 128:(s + 1) * 128],
                                wv_t[:, d, :],
                                start=(d == 0),
                                stop=(d == ND - 1),
                            )
                    for s in (2 * w, 2 * w + 1):
                        nc.vector.tensor_copy(
                            v_t[s][:, :, 0:DH],
                            ps[s][:].rearrange("p (h d) -> p h d", d=DH),
                        )
                        nc.vector.tensor_copy(v_t[s][:, :, DH], ones4[:])

                # (p0, hp0): attnV for the early heads, then normalize
                for hh in range(2):
                    av = av_block(0, 0, hh, ats_early[hh])
                    norm_block(0, 0, hh, av)

                def wo_block(p):
                    pc0 = p * PC
                    for im in range(ND):
                        for n in range(2):
                            wo_ps = ap_pool.tile(
                                [128, QC], f32, tag=f"sc{n}",
                                name=f"wo{im}_{n}_{p}",
                            )
                            for jk in range(2):
                                nc.tensor.matmul(
                                    wo_ps[:],
                                    wo_t[:, jk, im, :],
                                    ao_t[jk][:, pc0 + n * QC:
                                             pc0 + (n + 1) * QC],
                                    start=(jk == 0),
                                    stop=(jk == 1),
                                )
                            ot = oev_pool.tile([128, QC], f32, tag="ot",
                                               name=f"ot{im}_{n}_{p}")
                            nc.vector.tensor_copy(ot[:], wo_ps[:])
                            nc.sync.dma_start(
                                outT[im * 128:(im + 1) * 128,
                                     pc0 + n * QC:pc0 + (n + 1) * QC],
                                ot[:],
                            )

                # remaining (p, hp) combos; the out-projection for a
                # finished column pair is emitted just after the next
                # score block so its inputs are ready (no PE stall) and
                # the PE is still warm
                pending_wo = None
                for p, hp in ((0, 1), (1, 0), (1, 1)):
                    for hh in range(2):
                        ats = sc_exp_block(p, hp, hh)
                        if pending_wo is not None:
                            wo_block(pending_wo)
                            pending_wo = None
                        av = av_block(p, hp, hh, ats)
                        norm_block(p, hp, hh, av)
                    if hp == 1:
                        pending_wo = p
                wo_block(pending_wo)

    nc.compile()
    return nc


def _enable_ldw_opt():
    """Let walrus dedupe consecutive identical LDWEIGHTS (off by default
    in concourse; our inner loops reuse each stationary operand 2-4x)."""
    if _cache.get("ldw_patched"):
        return
    import concourse.bass_utils as bu

    orig = bu.run_command

    def patched(argv, **kw):
        argv = [
            "--enable-ldw-opt=true" if a == "--enable-ldw-opt=false" else a
            for a in argv
        ]
        return orig(argv, **kw)

    bu.run_command = patched
    _cache["ldw_patched"] = True


def _get_nc():
    if "nc" not in _cache:
        if int(os.environ.get("MHA_LDW_OPT", "0")):
            _enable_ldw_opt()
        _cache["nc"] = build_nc()
    return _cache["nc"]


def kernel(q, k, v, Wq, bq, Wk, bk, Wv, bv, Wo, bo, **_unused):
    import ml_dtypes
    from concourse.bass_utils import run_bass_kernel_spmd

    bf = ml_dtypes.bfloat16
    q = np.asarray(q, dtype=np.float32)
    k = np.asarray(k, dtype=np.float32)
    v = np.asarray(v, dtype=np.float32)
    Wq = np.asarray(Wq, dtype=np.float32)
    Wk = np.asarray(Wk, dtype=np.float32)
    Wv = np.asarray(Wv, dtype=np.float32)
    Wo = np.asarray(Wo, dtype=np.float32)
    bq = np.asarray(bq, dtype=np.float32)
    bk = np.asarray(bk, dtype=np.float32)
    bv = np.asarray(bv, dtype=np.float32)
    bo = np.asarray(bo, dtype=np.float32)

    nc = _get_nc()

    xT = {b: {} for b in range(B)}
    for b in range(B):
        xT[b]["q"] = np.ascontiguousarray(q[b].T).astype(bf)
        xT[b]["k"] = np.ascontiguousarray(k[b].T).astype(bf)
        xT[b]["v"] = np.ascontiguousarray(v[b].T).astype(bf)

    wslices = []
    for g in range(4):
        J = slice(g * JG, (g + 1) * JG)
        wslices.append({
            "wqT": np.ascontiguousarray(Wq.T[:, J]).astype(bf),
            "wkT": np.ascontiguousarray(Wk.T[:, J]).astype(bf),
            "wvT": np.ascontiguousarray(Wv.T[:, J]).astype(bf),
            "woT": np.ascontiguousarray(Wo[:, J].T).astype(bf),
            "bq": np.ascontiguousarray(bq[J]),
            "bk": np.ascontiguousarray(bk[J]),
        })

    in_maps = []
    for c in range(8):
        b, g = c // 4, c % 4
        m = {
            "xqT": xT[b]["q"], "xkT": xT[b]["k"], "xvT": xT[b]["v"],
        }
        m.update(wslices[g])
        in_maps.append(m)

    trace = bool(int(os.environ.get("KERNEL_TRACE", "0")))
    if trace:
        _install_profshim()
    res = run_bass_kernel_spmd(
        nc, in_maps, core_ids=list(range(8)), trace=trace
    )
    _cache["exec_time_ns"] = res.exec_time_ns
    parts = [r["outT"] for r in res.results]

    # host reduce: sum the 4 head-group partials per batch, transpose,
    # add the linear bias terms (bo + Wo @ bv, exact fold)
    const_row = bo + Wo @ bv
    out = np.empty((B, S, D), dtype=np.float32)
    for b in range(B):
        acc = parts[4 * b].copy()
        for g in range(1, 4):
            acc += parts[4 * b + g]
        out[b] = acc.T + const_row
    return out


# revision 29
# speedup vs baseline: 1.1228x; 1.1228x over previous
"""Multi-head attention (B=2, S=2048, D=1024, H=16) on 8 trn2 NeuronCores.

Sharding: core c handles batch b = c // 4 and head-group g = c % 4
(4 heads = 256 hidden columns per core).  Each core computes its 4 heads'
attention plus the partial out-projection; the host sums the 4 partials
per batch and adds the (linear) bias terms (bo + Wo @ bv) exactly.

v2: all matmuls in bf16 (1 cycle/row on the PE; fp32 accumulation in
PSUM), exp in [128,1024] tiles, reciprocal_approx_fast for softmax
denominators.

Layout per core (DRAM tensors bf16 unless noted):
  xqT, xkT, xvT : [1024, 2048]   x.T (host-transposed activations)
  wqT, wkT, wvT : [1024, 256]    W.T column slice for this head group
  woT           : [256, 1024]    Wo[:, J].T
  bq, bk        : [256]  fp32    bias slices (added via ACT during evac)
  outT (output) : [1024, 2048] fp32   partial (out @ Wo_J.T).T
"""
import os
import sys
import types

sys.path.insert(0, "/opt/trn_rl_repo")

import numpy as np

B = 2
S = 2048
D = 1024
H_PER_CORE = 4      # heads per core
DH = 64             # head dim
JG = 256            # hidden cols per core (4 heads * 64)
ND = D // 128       # 8 contraction d-tiles
NKT = S // 128      # 16 k-position tiles
QC = 512
PC = 1024           # processed q columns per pass (2 chunks of 512)
SCALE = 1.0 / np.sqrt(DH)

_cache = {}


def _install_profshim():
    """Enable NTFF profiling under axon (KERNEL_TRACE=1 only)."""
    if "antenv.axon_hooks" in sys.modules:
        return
    try:
        from trn_agent_boot.trn_boot import _ntff_profile_via_ctypes

        hook = _ntff_profile_via_ctypes("/opt/axon/libaxon_pjrt.so")
        mod = types.ModuleType("antenv.axon_hooks")
        mod.get_axon_ntff_profile_hook = lambda: hook
        mod.set_axon_ntff_profile_hook = lambda h: None
        sys.modules["antenv.axon_hooks"] = mod
        import concourse.bass_utils as _bu

        _bu.upload_artifacts = lambda tmpdir: "local://unavailable"
    except Exception:
        pass


def build_nc():
    import concourse.bacc as bacc
    import concourse.mybir as mybir
    import concourse.tile as tile

    f32 = mybir.dt.float32
    bf16 = mybir.dt.bfloat16
    AF = mybir.ActivationFunctionType

    nc = bacc.Bacc("TRN2", target_bir_lowering=False)

    xqT = nc.dram_tensor("xqT", [D, S], bf16, kind="ExternalInput").ap()
    xkT = nc.dram_tensor("xkT", [D, S], bf16, kind="ExternalInput").ap()
    xvT = nc.dram_tensor("xvT", [D, S], bf16, kind="ExternalInput").ap()
    wqT = nc.dram_tensor("wqT", [D, JG], bf16, kind="ExternalInput").ap()
    wkT = nc.dram_tensor("wkT", [D, JG], bf16, kind="ExternalInput").ap()
    wvT = nc.dram_tensor("wvT", [D, JG], bf16, kind="ExternalInput").ap()
    woT = nc.dram_tensor("woT", [JG, D], bf16, kind="ExternalInput").ap()
    bq = nc.dram_tensor("bq", [JG], f32, kind="ExternalInput").ap()
    bk = nc.dram_tensor("bk", [JG], f32, kind="ExternalInput").ap()
    outT = nc.dram_tensor("outT", [D, S], f32, kind="ExternalOutput").ap()

    with tile.TileContext(nc) as tc:
        with (
            tc.tile_pool(name="xt", bufs=8) as xt_pool,
            tc.tile_pool(name="wts", bufs=1) as w_pool,
            tc.tile_pool(name="qkv", bufs=1) as qkv_pool,
            tc.tile_pool(name="attn", bufs=2) as attn_pool,
            tc.tile_pool(name="small", bufs=1) as small_pool,
            tc.tile_pool(name="nrm", bufs=2) as nrm_pool,
            tc.tile_pool(name="oev", bufs=3) as oev_pool,
        ):
            # ---- weight / bias loads -------------------------------------
            # wq first so the very first projection matmul can start as
            # soon as wq + the first xq tile land; the rest follow behind
            wq_t = w_pool.tile([128, ND, JG], bf16, tag="wq")
            nc.sync.dma_start(wq_t[:], wqT.rearrange("(n p) j -> p n j", p=128))
            wk_t = w_pool.tile([128, ND, JG], bf16, tag="wk")
            wv_t = w_pool.tile([128, ND, JG], bf16, tag="wv")
            wo_t = w_pool.tile([128, 2, ND, 128], bf16, tag="wo")
            bq_t = small_pool.tile([128, 2], f32, tag="bq")
            bk_t = small_pool.tile([128, 2], f32, tag="bk")
            ones1 = small_pool.tile([1, DH], f32, tag="ones1")
            nc.vector.memset(ones1[:], 1.0)

            def load_rest_of_weights():
                nc.sync.dma_start(
                    wk_t[:], wkT.rearrange("(n p) j -> p n j", p=128)
                )
                nc.sync.dma_start(
                    wv_t[:], wvT.rearrange("(n p) j -> p n j", p=128)
                )
                nc.sync.dma_start(
                    wo_t[:],
                    woT.rearrange("(a p) (n m) -> p a n m", p=128, m=128),
                )
                nc.sync.dma_start(bq_t[:], bq.rearrange("(a p) -> p a", p=128))
                nc.sync.dma_start(bk_t[:], bk.rearrange("(a p) -> p a", p=128))

            # ---- persistent activation tensors ---------------------------
            q_t = [qkv_pool.tile([128, S], bf16, tag=f"qt{m}", name=f"qt{m}")
                   for m in range(2)]
            k_t = [qkv_pool.tile([128, S], bf16, tag=f"kt{m}", name=f"kt{m}")
                   for m in range(2)]
            # V (natural layout) + ones column per head: 16 s-tiles
            v_t = [qkv_pool.tile([128, H_PER_CORE, DH + 1], bf16,
                                 tag=f"v{s}", name=f"v{s}")
                   for s in range(NKT)]
            ao_t = [qkv_pool.tile([128, S], bf16, tag=f"ao{m}", name=f"ao{m}")
                    for m in range(2)]

            # ---- phase 1: projections (d-outer, PSUM-resident) -----------
            with tc.tile_pool(name="proj_psum", bufs=1, space="PSUM") as pp:
                for name, w_full, x_dram, dst, bias in (
                    ("q", wq_t, xqT, q_t, bq_t),
                    ("k", wk_t, xkT, k_t, bk_t),
                ):
                    xs = []
                    for d in range(ND):
                        xd = xt_pool.tile([128, S], bf16, tag="xT",
                                          name=f"x{name}{d}")
                        nc.sync.dma_start(
                            xd[:], x_dram[d * 128:(d + 1) * 128, :]
                        )
                        xs.append(xd)
                    if name == "q":
                        load_rest_of_weights()
                    ps = {
                        (m, c): pp.tile([128, QC], f32, tag=f"pp{m}{c}",
                                        name=f"ps{name}{m}{c}")
                        for m in range(2) for c in range(4)
                    }
                    for d in range(ND):
                        for m in range(2):
                            for c in range(4):
                                nc.tensor.matmul(
                                    ps[(m, c)][:],
                                    w_full[:, d, m * 128:(m + 1) * 128],
                                    xs[d][:, c * QC:(c + 1) * QC],
                                    start=(d == 0),
                                    stop=(d == ND - 1),
                                )
                    for m in range(2):
                        for c in range(4):
                            nc.vector.tensor_scalar_add(
                                dst[m][:, c * QC:(c + 1) * QC],
                                ps[(m, c)][:],
                                bias[:, m:m + 1],
                            )

                # xv tiles stream in behind the K loads
                xvs = []
                for d in range(ND):
                    xd = xt_pool.tile([128, S], bf16, tag="xT", name=f"xv{d}")
                    nc.sync.dma_start(xd[:], xvT[d * 128:(d + 1) * 128, :])
                    xvs.append(xd)

            # ---- phase 2+3: attention + V + out-projection ---------------
            # The first head-pair's score/exp blocks are emitted BEFORE the
            # V projection so the ACT exp queue drains while the PE runs V.
            with tc.tile_pool(name="apsum", bufs=1, space="PSUM") as ap_pool:

                def sc_exp_block(p, hp, hh):
                    pc0 = p * PC
                    po = hh * DH
                    ats = []
                    for kt in range(NKT):
                        sc_ps = ap_pool.tile(
                            [128, PC], f32, tag=f"sc{kt % 2}",
                            name=f"sc_{p}{hp}{hh}_{kt}",
                        )
                        for n in range(2):
                            nc.tensor.matmul(
                                sc_ps[:, n * QC:(n + 1) * QC],
                                k_t[hp][po:po + DH,
                                        kt * 128:(kt + 1) * 128],
                                q_t[hp][po:po + DH,
                                        pc0 + n * QC:pc0 + (n + 1) * QC],
                                start=True, stop=True,
                            )
                        at = attn_pool.tile(
                            [128, PC], bf16, tag=f"at{kt}",
                            name=f"at{p}{hp}{hh}_{kt}",
                        )
                        nc.scalar.activation(
                            at[:], sc_ps[:], AF.Exp, scale=float(SCALE)
                        )
                        ats.append(at)
                    return ats

                def av_block(p, hp, hh, ats):
                    h = hp * 2 + hh
                    av = ap_pool.tile(
                        [DH + 1, PC], f32, tag=f"av{hh}",
                        name=f"av{hh}_{hp}_{p}",
                    )
                    for kt in range(NKT):
                        for n in range(2):
                            nc.tensor.matmul(
                                av[:, n * QC:(n + 1) * QC],
                                v_t[kt][:, h, :],
                                ats[kt][:, n * QC:(n + 1) * QC],
                                start=(kt == 0),
                                stop=(kt == NKT - 1),
                            )
                    return av

                def norm_block(p, hp, hh, av):
                    # PE-free normalize: recip on DVE, partition broadcast
                    # on GPSIMD, multiply on DVE
                    psl = slice(p * PC, (p + 1) * PC)
                    po = hh * DH
                    dn = nrm_pool.tile([1, PC], f32, tag="dn",
                                       name=f"dn{p}{hp}{hh}")
                    nc.vector.tensor_copy(dn[:], av[DH:DH + 1, :])
                    rc = nrm_pool.tile([1, PC], f32, tag="rc",
                                       name=f"rc{p}{hp}{hh}")
                    nc.vector.reciprocal_approx_fast(rc[:], dn[:])
                    rb = nrm_pool.tile([DH, PC], f32, tag="rb",
                                       name=f"rb{p}{hp}{hh}")
                    nc.gpsimd.partition_broadcast(rb[:], rc[:])
                    nc.vector.tensor_mul(
                        ao_t[hp][po:po + DH, psl], av[0:DH, :], rb[:]
                    )

                # early scores for (p0, hp0) — fills the ACT pipe
                ats_early = {hh: sc_exp_block(0, 0, hh) for hh in range(2)}

                # V projection (PSUM slots borrowed from the av tags),
                # 8 waves of 2 s-tiles
                ones4 = small_pool.tile([128, H_PER_CORE], f32, tag="ones4")
                nc.vector.memset(ones4[:], 1.0)
                for w in range(8):
                    ps = {
                        s: ap_pool.tile([128, JG], f32, tag=f"av{s % 2}",
                                        name=f"psv{s}")
                        for s in (2 * w, 2 * w + 1)
                    }
                    for d in range(ND):
                        for s in (2 * w, 2 * w + 1):
                            nc.tensor.matmul(
                                ps[s][:],
                                xvs[d][:, s * 128:(s + 1) * 128],
                                wv_t[:, d, :],
                                start=(d == 0),
                                stop=(d == ND - 1),
                            )
                    for s in (2 * w, 2 * w + 1):
                        nc.vector.tensor_copy(
                            v_t[s][:, :, 0:DH],
                            ps[s][:].rearrange("p (h d) -> p h d", d=DH),
                        )
                        nc.vector.tensor_copy(v_t[s][:, :, DH], ones4[:])

                # (p0, hp0): attnV for the early heads, then normalize
                for hh in range(2):
                    av = av_block(0, 0, hh, ats_early[hh])
                    norm_block(0, 0, hh, av)

                def wo_block(p):
                    pc0 = p * PC
                    for im in range(ND):
                        for n in range(2):
                            wo_ps = ap_pool.tile(
                                [128, QC], f32, tag=f"sc{n}",
                                name=f"wo{im}_{n}_{p}",
                            )
                            for jk in range(2):
                                nc.tensor.matmul(
                                    wo_ps[:],
                                    wo_t[:, jk, im, :],
                                    ao_t[jk][:, pc0 + n * QC:
                                             pc0 + (n + 1) * QC],
                                    start=(jk == 0),
                                    stop=(jk == 1),
                                )
                            ot = oev_pool.tile([128, QC], f32, tag="ot",
                                               name=f"ot{im}_{n}_{p}")
                            nc.vector.tensor_copy(ot[:], wo_ps[:])
                            nc.sync.dma_start(
                                outT[im * 128:(im + 1) * 128,
                                     pc0 + n * QC:pc0 + (n + 1) * QC],
                                ot[:],
                            )

                # remaining (p, hp) combos in standard order
                for p, hp in ((0, 1), (1, 0), (1, 1)):
                    for hh in range(2):
                        ats = sc_exp_block(p, hp, hh)
                        av = av_block(p, hp, hh, ats)
                        norm_block(p, hp, hh, av)
                wo_block(0)
                wo_block(1)

    nc.compile()
    return nc


def _enable_ldw_opt():
    """Let walrus dedupe consecutive identical LDWEIGHTS (off by default
    in concourse; our inner loops reuse each stationary operand 2-4x)."""
    if _cache.get("ldw_patched"):
        return
    import concourse.bass_utils as bu

    orig = bu.run_command

    def patched(argv, **kw):
        argv = [
            "--enable-ldw-opt=true" if a == "--enable-ldw-opt=false" else a
            for a in argv
        ]
        return orig(argv, **kw)

    bu.run_command = patched
    _cache["ldw_patched"] = True


def _get_nc():
    if "nc" not in _cache:
        if int(os.environ.get("MHA_LDW_OPT", "0")):
            _enable_ldw_opt()
        _cache["nc"] = build_nc()
    return _cache["nc"]


def kernel(q, k, v, Wq, bq, Wk, bk, Wv, bv, Wo, bo, **_unused):
    import ml_dtypes
    from concourse.bass_utils import run_bass_kernel_spmd

    bf = ml_dtypes.bfloat16
    q = np.asarray(q, dtype=np.float32)
    k = np.asarray(k, dtype=np.float32)
    v = np.asarray(v, dtype=np.float32)
    Wq = np.asarray(Wq, dtype=np.float32)
    Wk = np.asarray(Wk, dtype=np.float32)
    Wv = np.asarray(Wv, dtype=np.float32)
    Wo = np.asarray(Wo, dtype=np.float32)
    bq = np.asarray(bq, dtype=np.float32)
    bk = np.asarray(bk, dtype=np.float32)
    bv = np.asarray(bv, dtype=np.float32)
    bo = np.asarray(bo, dtype=np.float32)

    nc = _get_nc()

    xT = {b: {} for b in range(B)}
    for b in range(B):
        xT[b]["q"] = np.ascontiguousarray(q[b].T).astype(bf)
        xT[b]["k"] = np.ascontiguousarray(k[b].T).astype(bf)
        xT[b]["v"] = np.ascontiguousarray(v[b].T).astype(bf)

    wslices = []
    for g in range(4):
        J = slice(g * JG, (g + 1) * JG)
        wslices.append({
            "wqT": np.ascontiguousarray(Wq.T[:, J]).astype(bf),
            "wkT": np.ascontiguousarray(Wk.T[:, J]).astype(bf),
            "wvT": np.ascontiguousarray(Wv.T[:, J]).astype(bf),
            "woT": np.ascontiguousarray(Wo[:, J].T).astype(bf),
            "bq": np.ascontiguousarray(bq[J]),
            "bk": np.ascontiguousarray(bk[J]),
        })

    in_maps = []
    for c in range(8):
        b, g = c // 4, c % 4
        m = {
            "xqT": xT[b]["q"], "xkT": xT[b]["k"], "xvT": xT[b]["v"],
        }
        m.update(wslices[g])
        in_maps.append(m)

    trace = bool(int(os.environ.get("KERNEL_TRACE", "0")))
    if trace:
        _install_profshim()
    res = run_bass_kernel_spmd(
        nc, in_maps, core_ids=list(range(8)), trace=trace
    )
    _cache["exec_time_ns"] = res.exec_time_ns
    parts = [r["outT"] for r in res.results]

    # host reduce: sum the 4 head-group partials per batch, transpose,
    # add the linear bias terms (bo + Wo @ bv, exact fold)
    const_row = bo + Wo @ bv
    out = np.empty((B, S, D), dtype=np.float32)
    for b in range(B):
        acc = parts[4 * b].copy()
        for g in range(1, 4):
            acc += parts[4 * b + g]
        out[b] = acc.T + const_row
    return out


# revision 30
# speedup vs baseline: 1.1331x; 1.0092x over previous
"""Multi-head attention (B=2, S=2048, D=1024, H=16) on 8 trn2 NeuronCores.

Sharding: core c handles batch b = c // 4 and head-group g = c % 4
(4 heads = 256 hidden columns per core).  Each core computes its 4 heads'
attention plus the partial out-projection; the host sums the 4 partials
per batch and adds the (linear) bias terms (bo + Wo @ bv) exactly.

v2: all matmuls in bf16 (1 cycle/row on the PE; fp32 accumulation in
PSUM), exp in [128,1024] tiles, reciprocal_approx_fast for softmax
denominators.

Layout per core (DRAM tensors bf16 unless noted):
  xqT, xkT, xvT : [1024, 2048]   x.T (host-transposed activations)
  wqT, wkT, wvT : [1024, 256]    W.T column slice for this head group
  woT           : [256, 1024]    Wo[:, J].T
  bq, bk        : [256]  fp32    bias slices (added via ACT during evac)
  outT (output) : [1024, 2048] fp32   partial (out @ Wo_J.T).T
"""
import os
import sys
import types

sys.path.insert(0, "/opt/trn_rl_repo")

import numpy as np

B = 2
S = 2048
D = 1024
H_PER_CORE = 4      # heads per core
DH = 64             # head dim
JG = 256            # hidden cols per core (4 heads * 64)
ND = D // 128       # 8 contraction d-tiles
NKT = S // 128      # 16 k-position tiles
QC = 512
PC = 1024           # processed q columns per pass (2 chunks of 512)
SCALE = 1.0 / np.sqrt(DH)

_cache = {}


def _install_profshim():
    """Enable NTFF profiling under axon (KERNEL_TRACE=1 only)."""
    if "antenv.axon_hooks" in sys.modules:
        return
    try:
        from trn_agent_boot.trn_boot import _ntff_profile_via_ctypes

        hook = _ntff_profile_via_ctypes("/opt/axon/libaxon_pjrt.so")
        mod = types.ModuleType("antenv.axon_hooks")
        mod.get_axon_ntff_profile_hook = lambda: hook
        mod.set_axon_ntff_profile_hook = lambda h: None
        sys.modules["antenv.axon_hooks"] = mod
        import concourse.bass_utils as _bu

        _bu.upload_artifacts = lambda tmpdir: "local://unavailable"
    except Exception:
        pass


def build_nc():
    import concourse.bacc as bacc
    import concourse.mybir as mybir
    import concourse.tile as tile

    f32 = mybir.dt.float32
    bf16 = mybir.dt.bfloat16
    AF = mybir.ActivationFunctionType

    nc = bacc.Bacc("TRN2", target_bir_lowering=False)

    xqT = nc.dram_tensor("xqT", [D, S], bf16, kind="ExternalInput").ap()
    xkT = nc.dram_tensor("xkT", [D, S], bf16, kind="ExternalInput").ap()
    xvT = nc.dram_tensor("xvT", [D, S], bf16, kind="ExternalInput").ap()
    wqT = nc.dram_tensor("wqT", [D, JG], bf16, kind="ExternalInput").ap()
    wkT = nc.dram_tensor("wkT", [D, JG], bf16, kind="ExternalInput").ap()
    wvT = nc.dram_tensor("wvT", [D, JG], bf16, kind="ExternalInput").ap()
    woT = nc.dram_tensor("woT", [JG, D], bf16, kind="ExternalInput").ap()
    bq = nc.dram_tensor("bq", [JG], f32, kind="ExternalInput").ap()
    bk = nc.dram_tensor("bk", [JG], f32, kind="ExternalInput").ap()
    outT = nc.dram_tensor("outT", [D, S], f32, kind="ExternalOutput").ap()

    with tile.TileContext(nc) as tc:
        with (
            tc.tile_pool(name="xt", bufs=10) as xt_pool,
            tc.tile_pool(name="wts", bufs=1) as w_pool,
            tc.tile_pool(name="qkv", bufs=1) as qkv_pool,
            tc.tile_pool(name="attn", bufs=2) as attn_pool,
            tc.tile_pool(name="small", bufs=1) as small_pool,
            tc.tile_pool(name="nrm", bufs=3) as nrm_pool,
            tc.tile_pool(name="oev", bufs=4) as oev_pool,
        ):
            # ---- weight / bias loads -------------------------------------
            # wq first so the very first projection matmul can start as
            # soon as wq + the first xq tile land; the rest follow behind
            wq_t = w_pool.tile([128, ND, JG], bf16, tag="wq")
            nc.sync.dma_start(wq_t[:], wqT.rearrange("(n p) j -> p n j", p=128))
            wk_t = w_pool.tile([128, ND, JG], bf16, tag="wk")
            wv_t = w_pool.tile([128, ND, JG], bf16, tag="wv")
            wo_t = w_pool.tile([128, 2, ND, 128], bf16, tag="wo")
            bq_t = small_pool.tile([128, 2], f32, tag="bq")
            bk_t = small_pool.tile([128, 2], f32, tag="bk")
            ones1 = small_pool.tile([1, DH], f32, tag="ones1")
            nc.vector.memset(ones1[:], 1.0)

            def load_rest_of_weights():
                nc.sync.dma_start(
                    wk_t[:], wkT.rearrange("(n p) j -> p n j", p=128)
                )
                nc.sync.dma_start(
                    wv_t[:], wvT.rearrange("(n p) j -> p n j", p=128)
                )
                nc.sync.dma_start(
                    wo_t[:],
                    woT.rearrange("(a p) (n m) -> p a n m", p=128, m=128),
                )
                nc.sync.dma_start(bq_t[:], bq.rearrange("(a p) -> p a", p=128))
                nc.sync.dma_start(bk_t[:], bk.rearrange("(a p) -> p a", p=128))

            # ---- persistent activation tensors ---------------------------
            q_t = [qkv_pool.tile([128, S], bf16, tag=f"qt{m}", name=f"qt{m}")
                   for m in range(2)]
            k_t = [qkv_pool.tile([128, S], bf16, tag=f"kt{m}", name=f"kt{m}")
                   for m in range(2)]
            # V (natural layout) + ones column per head: 16 s-tiles
            v_t = [qkv_pool.tile([128, H_PER_CORE, DH + 1], bf16,
                                 tag=f"v{s}", name=f"v{s}")
                   for s in range(NKT)]
            ao_t = [qkv_pool.tile([128, S], bf16, tag=f"ao{m}", name=f"ao{m}")
                    for m in range(2)]

            # ---- phase 1: projections (d-outer, PSUM-resident) -----------
            with tc.tile_pool(name="proj_psum", bufs=1, space="PSUM") as pp:
                for name, w_full, x_dram, dst, bias in (
                    ("q", wq_t, xqT, q_t, bq_t),
                    ("k", wk_t, xkT, k_t, bk_t),
                ):
                    xs = []
                    for d in range(ND):
                        xd = xt_pool.tile([128, S], bf16, tag="xT",
                                          name=f"x{name}{d}")
                        nc.sync.dma_start(
                            xd[:], x_dram[d * 128:(d + 1) * 128, :]
                        )
                        xs.append(xd)
                    if name == "q":
                        load_rest_of_weights()
                    ps = {
                        (m, c): pp.tile([128, QC], f32, tag=f"pp{m}{c}",
                                        name=f"ps{name}{m}{c}")
                        for m in range(2) for c in range(4)
                    }
                    for d in range(ND):
                        for m in range(2):
                            for c in range(4):
                                nc.tensor.matmul(
                                    ps[(m, c)][:],
                                    w_full[:, d, m * 128:(m + 1) * 128],
                                    xs[d][:, c * QC:(c + 1) * QC],
                                    start=(d == 0),
                                    stop=(d == ND - 1),
                                )
                    for m in range(2):
                        for c in range(4):
                            nc.vector.tensor_scalar_add(
                                dst[m][:, c * QC:(c + 1) * QC],
                                ps[(m, c)][:],
                                bias[:, m:m + 1],
                            )

                # xv tiles stream in behind the K loads
                xvs = []
                for d in range(ND):
                    xd = xt_pool.tile([128, S], bf16, tag="xT", name=f"xv{d}")
                    nc.sync.dma_start(xd[:], xvT[d * 128:(d + 1) * 128, :])
                    xvs.append(xd)

            # ---- phase 2+3: attention + V + out-projection ---------------
            # The first head-pair's score/exp blocks are emitted BEFORE the
            # V projection so the ACT exp queue drains while the PE runs V.
            with tc.tile_pool(name="apsum", bufs=1, space="PSUM") as ap_pool:

                def sc_exp_block(p, hp, hh):
                    pc0 = p * PC
                    po = hh * DH
                    ats = []
                    for kt in range(NKT):
                        sc_ps = ap_pool.tile(
                            [128, PC], f32, tag=f"sc{kt % 2}",
                            name=f"sc_{p}{hp}{hh}_{kt}",
                        )
                        for n in range(2):
                            nc.tensor.matmul(
                                sc_ps[:, n * QC:(n + 1) * QC],
                                k_t[hp][po:po + DH,
                                        kt * 128:(kt + 1) * 128],
                                q_t[hp][po:po + DH,
                                        pc0 + n * QC:pc0 + (n + 1) * QC],
                                start=True, stop=True,
                            )
                        at = attn_pool.tile(
                            [128, PC], bf16, tag=f"at{kt}",
                            name=f"at{p}{hp}{hh}_{kt}",
                        )
                        nc.scalar.activation(
                            at[:], sc_ps[:], AF.Exp, scale=float(SCALE)
                        )
                        ats.append(at)
                    return ats

                def av_block(p, hp, hh, ats):
                    h = hp * 2 + hh
                    av = ap_pool.tile(
                        [DH + 1, PC], f32, tag=f"av{hh}",
                        name=f"av{hh}_{hp}_{p}",
                    )
                    for kt in range(NKT):
                        for n in range(2):
                            nc.tensor.matmul(
                                av[:, n * QC:(n + 1) * QC],
                                v_t[kt][:, h, :],
                                ats[kt][:, n * QC:(n + 1) * QC],
                                start=(kt == 0),
                                stop=(kt == NKT - 1),
                            )
                    return av

                def norm_block(p, hp, hh, av):
                    # PE-free normalize: recip on DVE, partition broadcast
                    # on GPSIMD, multiply on DVE
                    psl = slice(p * PC, (p + 1) * PC)
                    po = hh * DH
                    dn = nrm_pool.tile([1, PC], f32, tag="dn",
                                       name=f"dn{p}{hp}{hh}")
                    nc.vector.tensor_copy(dn[:], av[DH:DH + 1, :])
                    rc = nrm_pool.tile([1, PC], f32, tag="rc",
                                       name=f"rc{p}{hp}{hh}")
                    nc.vector.reciprocal_approx_fast(rc[:], dn[:])
                    rb = nrm_pool.tile([DH, PC], f32, tag="rb",
                                       name=f"rb{p}{hp}{hh}")
                    nc.gpsimd.partition_broadcast(rb[:], rc[:])
                    nc.vector.tensor_mul(
                        ao_t[hp][po:po + DH, psl], av[0:DH, :], rb[:]
                    )

                # early scores for (p0, hp0) — fills the ACT pipe
                ats_early = {hh: sc_exp_block(0, 0, hh) for hh in range(2)}

                # V projection (PSUM slots borrowed from the av tags),
                # 8 waves of 2 s-tiles
                ones4 = small_pool.tile([128, H_PER_CORE], f32, tag="ones4")
                nc.vector.memset(ones4[:], 1.0)
                for w in range(8):
                    ps = {
                        s: ap_pool.tile([128, JG], f32, tag=f"av{s % 2}",
                                        name=f"psv{s}")
                        for s in (2 * w, 2 * w + 1)
                    }
                    for d in range(ND):
                        for s in (2 * w, 2 * w + 1):
                            nc.tensor.matmul(
                                ps[s][:],
                                xvs[d][:, s * 128:(s + 1) * 128],
                                wv_t[:, d, :],
                                start=(d == 0),
                                stop=(d == ND - 1),
                            )
                    for s in (2 * w, 2 * w + 1):
                        nc.vector.tensor_copy(
                            v_t[s][:, :, 0:DH],
                            ps[s][:].rearrange("p (h d) -> p h d", d=DH),
                        )
                        nc.vector.tensor_copy(v_t[s][:, :, DH], ones4[:])

                # (p0, hp0): attnV for the early heads, then normalize
                for hh in range(2):
                    av = av_block(0, 0, hh, ats_early[hh])
                    norm_block(0, 0, hh, av)

                def wo_block(p):
                    pc0 = p * PC
                    for im in range(ND):
                        for n in range(2):
                            wo_ps = ap_pool.tile(
                                [128, QC], f32, tag=f"sc{n}",
                                name=f"wo{im}_{n}_{p}",
                            )
                            for jk in range(2):
                                nc.tensor.matmul(
                                    wo_ps[:],
                                    wo_t[:, jk, im, :],
                                    ao_t[jk][:, pc0 + n * QC:
                                             pc0 + (n + 1) * QC],
                                    start=(jk == 0),
                                    stop=(jk == 1),
                                )
                            ot = oev_pool.tile([128, QC], f32, tag="ot",
                                               name=f"ot{im}_{n}_{p}")
                            nc.vector.tensor_copy(ot[:], wo_ps[:])
                            nc.sync.dma_start(
                                outT[im * 128:(im + 1) * 128,
                                     pc0 + n * QC:pc0 + (n + 1) * QC],
                                ot[:],
                            )

                # remaining (p, hp) combos in standard order
                for p, hp in ((0, 1), (1, 0), (1, 1)):
                    for hh in range(2):
                        ats = sc_exp_block(p, hp, hh)
                        av = av_block(p, hp, hh, ats)
                        norm_block(p, hp, hh, av)
                wo_block(0)
                wo_block(1)

    nc.compile()
    return nc


def _enable_ldw_opt():
    """Let walrus dedupe consecutive identical LDWEIGHTS (off by default
    in concourse; our inner loops reuse each stationary operand 2-4x)."""
    if _cache.get("ldw_patched"):
        return
    import concourse.bass_utils as bu

    orig = bu.run_command

    def patched(argv, **kw):
        argv = [
            "--enable-ldw-opt=true" if a == "--enable-ldw-opt=false" else a
            for a in argv
        ]
        return orig(argv, **kw)

    bu.run_command = patched
    _cache["ldw_patched"] = True


def _get_nc():
    if "nc" not in _cache:
        if int(os.environ.get("MHA_LDW_OPT", "0")):
            _enable_ldw_opt()
        _cache["nc"] = build_nc()
    return _cache["nc"]


def kernel(q, k, v, Wq, bq, Wk, bk, Wv, bv, Wo, bo, **_unused):
    import ml_dtypes
    from concourse.bass_utils import run_bass_kernel_spmd

    bf = ml_dtypes.bfloat16
    q = np.asarray(q, dtype=np.float32)
    k = np.asarray(k, dtype=np.float32)
    v = np.asarray(v, dtype=np.float32)
    Wq = np.asarray(Wq, dtype=np.float32)
    Wk = np.asarray(Wk, dtype=np.float32)
    Wv = np.asarray(Wv, dtype=np.float32)
    Wo = np.asarray(Wo, dtype=np.float32)
    bq = np.asarray(bq, dtype=np.float32)
    bk = np.asarray(bk, dtype=np.float32)
    bv = np.asarray(bv, dtype=np.float32)
    bo = np.asarray(bo, dtype=np.float32)

    nc = _get_nc()

    xT = {b: {} for b in range(B)}
    for b in range(B):
        xT[b]["q"] = np.ascontiguousarray(q[b].T).astype(bf)
        xT[b]["k"] = np.ascontiguousarray(k[b].T).astype(bf)
        xT[b]["v"] = np.ascontiguousarray(v[b].T).astype(bf)

    wslices = []
    for g in range(4):
        J = slice(g * JG, (g + 1) * JG)
        wslices.append({
            "wqT": np.ascontiguousarray(Wq.T[:, J]).astype(bf),
            "wkT": np.ascontiguousarray(Wk.T[:, J]).astype(bf),
            "wvT": np.ascontiguousarray(Wv.T[:, J]).astype(bf),
            "woT": np.ascontiguousarray(Wo[:, J].T).astype(bf),
            "bq": np.ascontiguousarray(bq[J]),
            "bk": np.ascontiguousarray(bk[J]),
        })

    in_maps = []
    for c in range(8):
        b, g = c // 4, c % 4
        m = {
            "xqT": xT[b]["q"], "xkT": xT[b]["k"], "xvT": xT[b]["v"],
        }
        m.update(wslices[g])
        in_maps.append(m)

    trace = bool(int(os.environ.get("KERNEL_TRACE", "0")))
    if trace:
        _install_profshim()
    res = run_bass_kernel_spmd(
        nc, in_maps, core_ids=list(range(8)), trace=trace
    )
    _cache["exec_time_ns"] = res.exec_time_ns
    parts = [r["outT"] for r in res.results]

    # host reduce: sum the 4 head-group partials per batch, transpose,
    # add the linear bias terms (bo + Wo @ bv, exact fold)
    const_row = bo + Wo @ bv
    out = np.empty((B, S, D), dtype=np.float32)
    for b in range(B):
        acc = parts[4 * b].copy()
        for g in range(1, 4):
            acc += parts[4 * b + g]
        out[b] = acc.T + const_row
    return out


# revision 31
# speedup vs baseline: 1.1805x; 1.0418x over previous
"""Multi-head attention (B=2, S=2048, D=1024, H=16) on 8 trn2 NeuronCores.

Sharding: core c handles batch b = c // 4 and head-group g = c % 4
(4 heads = 256 hidden columns per core).  Each core computes its 4 heads'
attention plus the partial out-projection; the host sums the 4 partials
per batch and adds the (linear) bias terms (bo + Wo @ bv) exactly.

All matmuls run in bf16 (1 cycle/row on the PE; fp32 accumulation in
PSUM).  Key structure, found via NTFF profiling:
  - activations are host-transposed to x.T so every PE contraction has
    d_model on partitions with line-rate contiguous DMA loads;
  - scores are computed transposed (scores_T = K_h^T-layout @ Q_h) so
    softmax's sum runs over PSUM partitions via a ones-column appended
    to V (denominator comes free out of the attnV matmul, row 64);
  - even/odd heads sit at partition base 0/64 so their K=64 score
    matmuls row-pack (run concurrently) in the PE array;
  - exp runs on ACT straight out of PSUM in [128,1024] tiles
    (scale=1/8 folded into the activation's affine input);
  - softmax normalization never touches the PE: reciprocal_approx_fast
    on DVE + partition_broadcast on the otherwise-idle GPSIMD;
  - the first head-pair's score/exp blocks are emitted before the V
    projection so the ACT exp pipe fills while the PE runs V;
  - out-projection consumes the transposed attention output directly
    (contraction over head columns), partials summed on host.

Measured on trn2 (8 cores): ~255 us HW exec, max rel err ~8e-3 vs
fp64 (bf16 input/weight rounding dominates; the f32r variant of this
kernel measured 4.7e-4 at ~2x the time — see kernel_f32r_v1.py).

Layout per core (DRAM tensors bf16 unless noted):
  xqT, xkT, xvT : [1024, 2048]   x.T (host-transposed activations)
  wqT, wkT, wvT : [1024, 256]    W.T column slice for this head group
  woT           : [256, 1024]    Wo[:, J].T
  bq, bk        : [256]  fp32    bias slices (added during PSUM evac)
  outT (output) : [1024, 2048] fp32   partial (out @ Wo_J.T).T
"""
import os
import sys
import types

sys.path.insert(0, "/opt/trn_rl_repo")

import numpy as np

B = 2
S = 2048
D = 1024
H_PER_CORE = 4      # heads per core
DH = 64             # head dim
JG = 256            # hidden cols per core (4 heads * 64)
ND = D // 128       # 8 contraction d-tiles
NKT = S // 128      # 16 k-position tiles
QC = 512
PC = 1024           # processed q columns per pass (2 chunks of 512)
SCALE = 1.0 / np.sqrt(DH)

_cache = {}


def _install_profshim():
    """Enable NTFF profiling under axon (KERNEL_TRACE=1 only)."""
    if "antenv.axon_hooks" in sys.modules:
        return
    try:
        from trn_agent_boot.trn_boot import _ntff_profile_via_ctypes

        hook = _ntff_profile_via_ctypes("/opt/axon/libaxon_pjrt.so")
        mod = types.ModuleType("antenv.axon_hooks")
        mod.get_axon_ntff_profile_hook = lambda: hook
        mod.set_axon_ntff_profile_hook = lambda h: None
        sys.modules["antenv.axon_hooks"] = mod
        import concourse.bass_utils as _bu

        _bu.upload_artifacts = lambda tmpdir: "local://unavailable"
    except Exception:
        pass


def build_nc():
    import concourse.bacc as bacc
    import concourse.mybir as mybir
    import concourse.tile as tile

    f32 = mybir.dt.float32
    bf16 = mybir.dt.bfloat16
    AF = mybir.ActivationFunctionType

    nc = bacc.Bacc("TRN2", target_bir_lowering=False)

    xqT = nc.dram_tensor("xqT", [D, S], bf16, kind="ExternalInput").ap()
    xkT = nc.dram_tensor("xkT", [D, S], bf16, kind="ExternalInput").ap()
    xvT = nc.dram_tensor("xvT", [D, S], bf16, kind="ExternalInput").ap()
    wqT = nc.dram_tensor("wqT", [D, JG], bf16, kind="ExternalInput").ap()
    wkT = nc.dram_tensor("wkT", [D, JG], bf16, kind="ExternalInput").ap()
    wvT = nc.dram_tensor("wvT", [D, JG], bf16, kind="ExternalInput").ap()
    woT = nc.dram_tensor("woT", [JG, D], bf16, kind="ExternalInput").ap()
    bq = nc.dram_tensor("bq", [JG], f32, kind="ExternalInput").ap()
    bk = nc.dram_tensor("bk", [JG], f32, kind="ExternalInput").ap()
    outT = nc.dram_tensor("outT", [D, S], f32, kind="ExternalOutput").ap()

    with tile.TileContext(nc) as tc:
        with (
            tc.tile_pool(name="xt", bufs=10) as xt_pool,
            tc.tile_pool(name="wts", bufs=1) as w_pool,
            tc.tile_pool(name="qkv", bufs=1) as qkv_pool,
            tc.tile_pool(name="attn", bufs=2) as attn_pool,
            tc.tile_pool(name="small", bufs=1) as small_pool,
            tc.tile_pool(name="nrm", bufs=3) as nrm_pool,
            tc.tile_pool(name="oev", bufs=4) as oev_pool,
        ):
            # ---- weight / bias loads -------------------------------------
            # wq first so the very first projection matmul can start as
            # soon as wq + the first xq tile land; the rest follow behind
            wq_t = w_pool.tile([128, ND, JG], bf16, tag="wq")
            nc.sync.dma_start(wq_t[:], wqT.rearrange("(n p) j -> p n j", p=128))
            wk_t = w_pool.tile([128, ND, JG], bf16, tag="wk")
            wv_t = w_pool.tile([128, ND, JG], bf16, tag="wv")
            wo_t = w_pool.tile([128, 2, ND, 128], bf16, tag="wo")
            bq_t = small_pool.tile([128, 2], f32, tag="bq")
            bk_t = small_pool.tile([128, 2], f32, tag="bk")
            ones1 = small_pool.tile([1, DH], f32, tag="ones1")
            nc.vector.memset(ones1[:], 1.0)

            def load_rest_of_weights():
                nc.sync.dma_start(
                    wk_t[:], wkT.rearrange("(n p) j -> p n j", p=128)
                )
                nc.sync.dma_start(
                    wv_t[:], wvT.rearrange("(n p) j -> p n j", p=128)
                )
                nc.sync.dma_start(
                    wo_t[:],
                    woT.rearrange("(a p) (n m) -> p a n m", p=128, m=128),
                )
                nc.sync.dma_start(bq_t[:], bq.rearrange("(a p) -> p a", p=128))
                nc.sync.dma_start(bk_t[:], bk.rearrange("(a p) -> p a", p=128))

            # ---- persistent activation tensors ---------------------------
            q_t = [qkv_pool.tile([128, S], bf16, tag=f"qt{m}", name=f"qt{m}")
                   for m in range(2)]
            k_t = [qkv_pool.tile([128, S], bf16, tag=f"kt{m}", name=f"kt{m}")
                   for m in range(2)]
            # V (natural layout) + ones column per head: 16 s-tiles
            v_t = [qkv_pool.tile([128, H_PER_CORE, DH + 1], bf16,
                                 tag=f"v{s}", name=f"v{s}")
                   for s in range(NKT)]
            ao_t = [qkv_pool.tile([128, S], bf16, tag=f"ao{m}", name=f"ao{m}")
                    for m in range(2)]

            # ---- phase 1: projections (d-outer, PSUM-resident) -----------
            with tc.tile_pool(name="proj_psum", bufs=1, space="PSUM") as pp:
                for name, w_full, x_dram, dst, bias in (
                    ("q", wq_t, xqT, q_t, bq_t),
                    ("k", wk_t, xkT, k_t, bk_t),
                ):
                    xs = []
                    for d in range(ND):
                        xd = xt_pool.tile([128, S], bf16, tag="xT",
                                          name=f"x{name}{d}")
                        nc.sync.dma_start(
                            xd[:], x_dram[d * 128:(d + 1) * 128, :]
                        )
                        xs.append(xd)
                    if name == "q":
                        load_rest_of_weights()
                    ps = {
                        (m, c): pp.tile([128, QC], f32, tag=f"pp{m}{c}",
                                        name=f"ps{name}{m}{c}")
                        for m in range(2) for c in range(4)
                    }
                    for d in range(ND):
                        for m in range(2):
                            for c in range(4):
                                nc.tensor.matmul(
                                    ps[(m, c)][:],
                                    w_full[:, d, m * 128:(m + 1) * 128],
                                    xs[d][:, c * QC:(c + 1) * QC],
                                    start=(d == 0),
                                    stop=(d == ND - 1),
                                )
                    for m in range(2):
                        for c in range(4):
                            nc.vector.tensor_scalar_add(
                                dst[m][:, c * QC:(c + 1) * QC],
                                ps[(m, c)][:],
                                bias[:, m:m + 1],
                            )

                # xv tiles stream in behind the K loads
                xvs = []
                for d in range(ND):
                    xd = xt_pool.tile([128, S], bf16, tag="xT", name=f"xv{d}")
                    nc.sync.dma_start(xd[:], xvT[d * 128:(d + 1) * 128, :])
                    xvs.append(xd)

            # ---- phase 2+3: attention + V + out-projection ---------------
            # The first head-pair's score/exp blocks are emitted BEFORE the
            # V projection so the ACT exp queue drains while the PE runs V.
            with tc.tile_pool(name="apsum", bufs=1, space="PSUM") as ap_pool:

                def sc_exp_block(p, hp, hh):
                    pc0 = p * PC
                    po = hh * DH
                    ats = []
                    for kt in range(NKT):
                        sc_ps = ap_pool.tile(
                            [128, PC], f32, tag=f"sc{kt % 2}",
                            name=f"sc_{p}{hp}{hh}_{kt}",
                        )
                        for n in range(2):
                            nc.tensor.matmul(
                                sc_ps[:, n * QC:(n + 1) * QC],
                                k_t[hp][po:po + DH,
                                        kt * 128:(kt + 1) * 128],
                                q_t[hp][po:po + DH,
                                        pc0 + n * QC:pc0 + (n + 1) * QC],
                                start=True, stop=True,
                            )
                        at = attn_pool.tile(
                            [128, PC], bf16, tag=f"at{kt}",
                            name=f"at{p}{hp}{hh}_{kt}",
                        )
                        nc.scalar.activation(
                            at[:], sc_ps[:], AF.Exp, scale=float(SCALE)
                        )
                        ats.append(at)
                    return ats

                def av_block(p, hp, hh, ats):
                    h = hp * 2 + hh
                    av = ap_pool.tile(
                        [DH + 1, PC], f32, tag=f"av{hh}",
                        name=f"av{hh}_{hp}_{p}",
                    )
                    for kt in range(NKT):
                        for n in range(2):
                            nc.tensor.matmul(
                                av[:, n * QC:(n + 1) * QC],
                                v_t[kt][:, h, :],
                                ats[kt][:, n * QC:(n + 1) * QC],
                                start=(kt == 0),
                                stop=(kt == NKT - 1),
                            )
                    return av

                def norm_block(p, hp, hh, av):
                    # PE-free normalize: recip on DVE, partition broadcast
                    # on GPSIMD, multiply on DVE
                    psl = slice(p * PC, (p + 1) * PC)
                    po = hh * DH
                    dn = nrm_pool.tile([1, PC], f32, tag="dn",
                                       name=f"dn{p}{hp}{hh}")
                    nc.vector.tensor_copy(dn[:], av[DH:DH + 1, :])
                    rc = nrm_pool.tile([1, PC], f32, tag="rc",
                                       name=f"rc{p}{hp}{hh}")
                    nc.vector.reciprocal_approx_fast(rc[:], dn[:])
                    rb = nrm_pool.tile([DH, PC], f32, tag="rb",
                                       name=f"rb{p}{hp}{hh}")
                    nc.gpsimd.partition_broadcast(rb[:], rc[:])
                    nc.vector.tensor_mul(
                        ao_t[hp][po:po + DH, psl], av[0:DH, :], rb[:]
                    )

                # early scores for (p0, hp0) — fills the ACT pipe
                ats_early = {hh: sc_exp_block(0, 0, hh) for hh in range(2)}

                # V projection (PSUM slots borrowed from the av tags),
                # 8 waves of 2 s-tiles
                ones4 = small_pool.tile([128, H_PER_CORE], f32, tag="ones4")
                nc.vector.memset(ones4[:], 1.0)
                for w in range(8):
                    ps = {
                        s: ap_pool.tile([128, JG], f32, tag=f"av{s % 2}",
                                        name=f"psv{s}")
                        for s in (2 * w, 2 * w + 1)
                    }
                    for d in range(ND):
                        for s in (2 * w, 2 * w + 1):
                            nc.tensor.matmul(
                                ps[s][:],
                                xvs[d][:, s * 128:(s + 1) * 128],
                                wv_t[:, d, :],
                                start=(d == 0),
                                stop=(d == ND - 1),
                            )
                    for s in (2 * w, 2 * w + 1):
                        nc.vector.tensor_copy(
                            v_t[s][:, :, 0:DH],
                            ps[s][:].rearrange("p (h d) -> p h d", d=DH),
                        )
                        nc.vector.tensor_copy(v_t[s][:, :, DH], ones4[:])

                # (p0, hp0): attnV for the early heads, then normalize
                for hh in range(2):
                    av = av_block(0, 0, hh, ats_early[hh])
                    norm_block(0, 0, hh, av)

                def wo_block(p):
                    pc0 = p * PC
                    for im in range(ND):
                        for n in range(2):
                            wo_ps = ap_pool.tile(
                                [128, QC], f32, tag=f"sc{n}",
                                name=f"wo{im}_{n}_{p}",
                            )
                            for jk in range(2):
                                nc.tensor.matmul(
                                    wo_ps[:],
                                    wo_t[:, jk, im, :],
                                    ao_t[jk][:, pc0 + n * QC:
                                             pc0 + (n + 1) * QC],
                                    start=(jk == 0),
                                    stop=(jk == 1),
                                )
                            ot = oev_pool.tile([128, QC], f32, tag="ot",
                                               name=f"ot{im}_{n}_{p}")
                            nc.vector.tensor_copy(ot[:], wo_ps[:])
                            nc.sync.dma_start(
                                outT[im * 128:(im + 1) * 128,
                                     pc0 + n * QC:pc0 + (n + 1) * QC],
                                ot[:],
                            )

                # remaining (p, hp) combos in standard order
                for p, hp in ((0, 1), (1, 0), (1, 1)):
                    for hh in range(2):
                        ats = sc_exp_block(p, hp, hh)
                        av = av_block(p, hp, hh, ats)
                        norm_block(p, hp, hh, av)
                wo_block(0)
                wo_block(1)

    nc.compile()
    return nc


def _enable_ldw_opt():
    """Let walrus dedupe consecutive identical LDWEIGHTS (off by default
    in concourse; our inner loops reuse each stationary operand 2-4x)."""
    if _cache.get("ldw_patched"):
        return
    import concourse.bass_utils as bu

    orig = bu.run_command

    def patched(argv, **kw):
        argv = [
            "--enable-ldw-opt=true" if a == "--enable-ldw-opt=false" else a
            for a in argv
        ]
        return orig(argv, **kw)

    bu.run_command = patched
    _cache["ldw_patched"] = True


def _get_nc():
    if "nc" not in _cache:
        if int(os.environ.get("MHA_LDW_OPT", "0")):
            _enable_ldw_opt()
        _cache["nc"] = build_nc()
    return _cache["nc"]


def kernel(q, k, v, Wq, bq, Wk, bk, Wv, bv, Wo, bo, **_unused):
    import ml_dtypes
    from concourse.bass_utils import run_bass_kernel_spmd

    bf = ml_dtypes.bfloat16
    q = np.asarray(q, dtype=np.float32)
    k = np.asarray(k, dtype=np.float32)
    v = np.asarray(v, dtype=np.float32)
    Wq = np.asarray(Wq, dtype=np.float32)
    Wk = np.asarray(Wk, dtype=np.float32)
    Wv = np.asarray(Wv, dtype=np.float32)
    Wo = np.asarray(Wo, dtype=np.float32)
    bq = np.asarray(bq, dtype=np.float32)
    bk = np.asarray(bk, dtype=np.float32)
    bv = np.asarray(bv, dtype=np.float32)
    bo = np.asarray(bo, dtype=np.float32)

    nc = _get_nc()

    xT = {b: {} for b in range(B)}
    for b in range(B):
        xT[b]["q"] = np.ascontiguousarray(q[b].T).astype(bf)
        xT[b]["k"] = np.ascontiguousarray(k[b].T).astype(bf)
        xT[b]["v"] = np.ascontiguousarray(v[b].T).astype(bf)

    wslices = []
    for g in range(4):
        J = slice(g * JG, (g + 1) * JG)
        wslices.append({
            "wqT": np.ascontiguousarray(Wq.T[:, J]).astype(bf),
            "wkT": np.ascontiguousarray(Wk.T[:, J]).astype(bf),
            "wvT": np.ascontiguousarray(Wv.T[:, J]).astype(bf),
            "woT": np.ascontiguousarray(Wo[:, J].T).astype(bf),
            "bq": np.ascontiguousarray(bq[J]),
            "bk": np.ascontiguousarray(bk[J]),
        })

    in_maps = []
    for c in range(8):
        b, g = c // 4, c % 4
        m = {
            "xqT": xT[b]["q"], "xkT": xT[b]["k"], "xvT": xT[b]["v"],
        }
        m.update(wslices[g])
        in_maps.append(m)

    trace = bool(int(os.environ.get("KERNEL_TRACE", "0")))
    if trace:
        _install_profshim()
    res = run_bass_kernel_spmd(
        nc, in_maps, core_ids=list(range(8)), trace=trace
    )
    _cache["exec_time_ns"] = res.exec_time_ns
    parts = [r["outT"] for r in res.results]

    # host reduce: sum the 4 head-group partials per batch, transpose,
    # add the linear bias terms (bo + Wo @ bv, exact fold)
    const_row = bo + Wo @ bv
    out = np.empty((B, S, D), dtype=np.float32)
    for b in range(B):
        acc = parts[4 * b].copy()
        for g in range(1, 4):
            acc += parts[4 * b + g]
        out[b] = acc.T + const_row
    return out
